# revision 1
# baseline (speedup 1.0000x reference)
"""Trainium2 Bass kernel for nn_LogicConv3d (differentiable-logic conv tree).

Problem (hardcoded): x [16,64,32,32] f32; idx_a/idx_b [64,900,64,3] i32;
w0..w6 [s,64,16] f32 (s = 64,32,16,8,4,2,1). Output [16,64,900,1] f32.

Math: per (kernel k, window p): gather 64 (a,b) leaf pairs from x, blend each
pair with soft-gate coefficients (softmax(w) @ GATE_M), then 6 more pairwise
tree levels.  mix(a,b) = c0 + c1*a + c2*b + c3*a*b.

Mapping:
 - F-sharding: core i handles batches (2i, 2i+1); all 64 kernels on every core
   -> the device program and all tables are identical across cores (pure SPMD);
   only the x-slice differs.
 - Indices are separable: idx[k,p,s] = (ha+hh_p, wa+ww_p, ca) so the leaf
   gather per (k,s) is a 30x30 crop of image x[b, ca] at (ha, wa).  The two
   batches are interleaved host-side (xsrc[c,h,w,b]) so ONE 1920-float
   consecutive run starting at (ca*1024 + ha*32 + wa)*2 contains the whole
   crop for both batches at positions 64*r + 2*q + b (q<30).  One
   indirect-DMA index per SBUF partition row fetches it at full bandwidth.
 - Tree levels run on-chip: partition dim = (node-msb, kernel), free =
   (window, batch).  Tiles at each level are keyed by the low bits of the
   node index so every merge op reads two full tiles at equal partition
   bases (HW constraint).
 - mix is 3 ops: ACT: p = c3*a + c2'; DVE stt: q = (b + beta) * p;
   DVE stt: r = (a * c1) + q; the additive constant c0 (+c1*alpha) folds into
   the next level's scalars (tree edges are single-use), added once at the end.
"""
import numpy as np

B, C, H, W = 16, 64, 32, 32
K = 64
RF = 3
DEPTH = 6
S = 64
PW = 30            # windows per axis
P = PW * PW        # 900
NCORES = 8
B2 = 2             # batches per core
F = P * B2         # free size (windows x batches)
XPAD = 131088      # 2*C*H*W + 16 pad (gather tail can run 4 past the end)

GATE_M = np.array([
    [0, 0, 0, 0], [0, 0, 0, 1], [0, 1, 0, -1], [0, 1, 0, 0],
    [0, 0, 1, -1], [0, 0, 1, 0], [0, 1, 1, -2], [0, 1, 1, -1],
    [1, -1, -1, 1], [1, -1, -1, 2], [1, 0, -1, 0], [1, 0, -1, 1],
    [1, -1, 0, 0], [1, -1, 0, 1], [1, 0, 0, -1], [1, 0, 0, 0],
], dtype=np.float32)  # [16 gates, 4] -> c0,c1,c2,c3 = GATE_M.T @ softmax(w)


# ---------------------------------------------------------------------------
# static schedule: the merge-tree op list
# ---------------------------------------------------------------------------
def _build_schedule():
    """Each mix op: dict(level, key, lanes, base, node[lanes], kern[lanes]).
    L0 ops read gather tiles A_key/B_key; level l>=1 ops read T_{l-1}[2k],[2k+1].
    DFS order keeps the live tile set small."""
    ops = []

    def emit(l, key):
        if l == 0:
            lanes = np.arange(128)
            ops.append(dict(level=0, key=key, lanes=128, base=0,
                            node=key + 32 * (lanes >> 6), kern=lanes & 63))
            return
        emit(l - 1, 2 * key)
        emit(l - 1, 2 * key + 1)
        lanes = np.arange(128)
        nbits_out = 6 - l
        ops.append(dict(level=l, key=key, lanes=128, base=0,
                        node=((lanes >> 6) << (nbits_out - 1)) + key,
                        kern=lanes & 63))

    emit(4, 0)
    emit(4, 1)
    # L5: one full op; node i5 = lane>>6 (a DMA then realigns the top half
    # to a base-0 tile for L6's equal-base inputs)
    lanes = np.arange(128)
    ops.append(dict(level=5, key=0, lanes=128, base=0,
                    node=lanes >> 6, kern=lanes & 63))
    lanes = np.arange(64)
    ops.append(dict(level=6, key=0, lanes=64, base=0,
                    node=np.zeros(64, np.int64), kern=lanes))
    return ops


_SCHED = _build_schedule()
_NMIX = len(_SCHED)          # 65
_NCOLS = 4 * _NMIX + 4       # + final gamma column block


def _softmax_f32(w):
    w = w.astype(np.float64)
    m = w.max(-1, keepdims=True)
    e = np.exp(w - m)
    return e / e.sum(-1, keepdims=True)


def _coef_tables(ws):
    """ws = [w0..w6]. Returns coef matrix [128, _NCOLS] f32 with per-op scalar
    columns (c3, bias, beta, c1) and the final gamma column."""
    cs = []
    for wl in ws:
        p = _softmax_f32(wl)                      # [s, K, 16] f64
        cs.append(np.einsum('skg,gj->skj', p, GATE_M.astype(np.float64)))
    gamma = [None] * 7
    gamma[0] = cs[0][:, :, 0]                     # c0, alpha=0 at leaves
    for l in range(1, 7):
        gamma[l] = cs[l][:, :, 0] + cs[l][:, :, 1] * gamma[l - 1][0::2]
    coef = np.zeros((128, _NCOLS), dtype=np.float64)
    for i, op in enumerate(_SCHED):
        l, node, kern = op['level'], op['node'], op['kern']
        rows = op['base'] + np.arange(op['lanes'])
        c = cs[l][node, kern]                     # [lanes, 4]
        if l == 0:
            alpha = np.zeros(op['lanes'])
            beta = np.zeros(op['lanes'])
        else:
            alpha = gamma[l - 1][2 * node, kern]
            beta = gamma[l - 1][2 * node + 1, kern]
        coef[rows, 4 * i + 0] = c[:, 3]                      # ACT scale = c3
        coef[rows, 4 * i + 1] = c[:, 2] + alpha * c[:, 3]    # ACT bias
        coef[rows, 4 * i + 2] = beta                         # stt1 scalar
        coef[rows, 4 * i + 3] = c[:, 1]                      # stt2 scalar = c1
    coef[0:64, 4 * _NMIX] = gamma[6][0, :]                   # final add
    return coef.astype(np.float32)


def _offset_tables(idx_a, idx_b):
    """Gather index tables [128, 64] i32: col = 2*t + side.
    Element offsets into the b-interleaved x-slice."""
    offs = np.zeros((128, 64), dtype=np.int64)
    for op in _SCHED:
        if op['level'] != 0:
            continue
        t = op['key']
        for side, idx in ((0, idx_a), (1, idx_b)):
            ha = idx[op['kern'], 0, op['node'], 0].astype(np.int64)
            wa = idx[op['kern'], 0, op['node'], 1].astype(np.int64)
            ca = idx[op['kern'], 0, op['node'], 2].astype(np.int64)
            offs[:, 2 * t + side] = (ca * (H * W) + ha * W + wa) * B2
    return offs.astype(np.int32)


# ---------------------------------------------------------------------------
# numpy emulator (mirrors the device schedule exactly; for validation)
# ---------------------------------------------------------------------------
def _emulate_core(xp, offs, coef):
    """xp: [XPAD] f32 b-interleaved slice. Returns [64, F] f32 (hh,ww,b)."""
    tiles = {}
    for i, op in enumerate(_SCHED):
        l, key, n, base = op['level'], op['key'], op['lanes'], op['base']
        rows = base + np.arange(n)
        sc = coef[rows, 4 * i + 0][:, None]
        bi = coef[rows, 4 * i + 1][:, None]
        be = coef[rows, 4 * i + 2][:, None]
        c1 = coef[rows, 4 * i + 3][:, None]
        if l == 0:
            ab = []
            for side in (0, 1):
                o = offs[:, 2 * key + side]
                raw = xp[o[:, None] + np.arange(1920)[None, :]]
                ab.append(raw.reshape(128, 30, 32, 2)[:, :, :30, :]
                          .reshape(128, F))
            a, b = ab
        elif l < 5:
            a = tiles[(l - 1, 2 * key)]
            b = tiles[(l - 1, 2 * key + 1)]
        elif l == 5:
            a = tiles[(4, 0)]
            b = tiles[(4, 1)]
        else:
            a = tiles['T5'][0:64]
            b = tiles['T5'][64:128]
        p = a * sc + bi
        q = (b + be) * p
        r = a * c1 + q
        if l == 5:
            tiles['T5'] = r
        else:
            tiles[(l, key)] = r
    return tiles[(6, 0)]


# ---------------------------------------------------------------------------
# Bass program (built once, cached)
# ---------------------------------------------------------------------------
_BASS_CACHE = {}


def _build_bass():
    if 'nc' in _BASS_CACHE:
        return _BASS_CACHE['nc']
    import concourse.bass as bass
    import concourse.mybir as mybir
    import concourse.tile as tile
    import concourse.bacc as bacc

    f32 = mybir.dt.float32
    nc = bacc.Bacc("TRN2", target_bir_lowering=False, debug=False,
                   num_devices=NCORES)
    xsrc_d = nc.dram_tensor("xsrc", [XPAD, 1], f32, kind="ExternalInput").ap()
    offs_d = nc.dram_tensor("offs", [128, 64], mybir.dt.int32,
                            kind="ExternalInput").ap()
    coef_d = nc.dram_tensor("coef", [128, _NCOLS], f32,
                            kind="ExternalInput").ap()
    out_d = nc.dram_tensor("out", [64, F], f32, kind="ExternalOutput").ap()

    AL = mybir.AluOpType
    ACTF = mybir.ActivationFunctionType

    def raw_view(t):      # [128,1920] -> [128,30,30,2] strided (skip w=30,31)
        return t[:].rearrange("p (h w b) -> p h w b",
                              h=30, w=32, b=2)[:, :, 0:30, :]

    def shp(x):           # compact [n,1800] AP -> [n,30,30,2]
        return x.rearrange("p (h w b) -> p h w b", h=30, w=30, b=2)

    with tile.TileContext(nc) as tc:
        with (
            tc.tile_pool(name="const", bufs=1) as pc,
            tc.tile_pool(name="ab", bufs=4) as pab,
            tc.tile_pool(name="lvl", bufs=2) as plv,
            tc.tile_pool(name="t0p", bufs=2) as pt0,
            tc.tile_pool(name="tmp", bufs=6) as ptmp,
            tc.tile_pool(name="fin", bufs=1) as pfin,
        ):
            offs_t = pc.tile([128, 64], mybir.dt.int32, tag="offs",
                             name="offs_t")
            nc.gpsimd.dma_start(offs_t[:], offs_d[:])
            coef_t = pc.tile([128, _NCOLS], f32, tag="coef", name="coef_t")
            nc.sync.dma_start(coef_t[:], coef_d[:])
            warm_t = pc.tile([1, 8], f32, tag="warm", name="warm_t")
            nc.scalar.activation(warm_t[:], coef_t[0:1, 0:8],
                                 ACTF.Identity, bias=0.0, scale=1.0)

            tiles = {}
            for i, op in enumerate(_SCHED):
                l, key, n, base = op['level'], op['key'], op['lanes'], op['base']
                sl = slice(base, base + n)
                sc = coef_t[sl, 4 * i + 0:4 * i + 1]
                bi = coef_t[sl, 4 * i + 1:4 * i + 2]
                be = coef_t[sl, 4 * i + 2:4 * i + 3]
                c1 = coef_t[sl, 4 * i + 3:4 * i + 4]
                if l == 0 and key == 0:
                    # first triple: gather + compute in h-halves so the DVE
                    # stream starts ~4us earlier (smaller first transfers)
                    r_t = pt0.tile([128, F], f32, tag="T0", name="t0_0")
                    tiles[(0, 0)] = r_t
                    for half in (0, 1):
                        ah = pab.tile([128, 960], f32, tag="A", name="at")
                        bh = pab.tile([128, 960], f32, tag="B", name="bt")
                        for side, dst in ((0, ah), (1, bh)):
                            nc.gpsimd.indirect_dma_start(
                                out=dst[:], out_offset=None, in_=xsrc_d[:],
                                in_offset=bass.IndirectOffsetOnAxis(
                                    ap=offs_t[:, side:side + 1], axis=0),
                                element_offset=960 * half)
                        av = ah[:].rearrange("p (h w b) -> p h w b",
                                             h=15, w=32, b=2)[:, :, 0:30, :]
                        bv = bh[:].rearrange("p (h w b) -> p h w b",
                                             h=15, w=32, b=2)[:, :, 0:30, :]
                        ph = ptmp.tile([128, F], f32, tag="p", name="p")
                        phv = ph[:, 0:900].rearrange(
                            "p (h w b) -> p h w b", h=15, w=30, b=2)
                        nc.scalar.activation(phv, av, ACTF.Identity,
                                             bias=bi, scale=sc)
                        nc.vector.scalar_tensor_tensor(
                            out=phv, in0=bv, scalar=be, in1=phv,
                            op0=AL.add, op1=AL.mult)
                        rhv = shp(r_t[:])[:, 15 * half:15 * half + 15, :, :]
                        nc.vector.scalar_tensor_tensor(
                            out=rhv, in0=av, scalar=c1, in1=phv,
                            op0=AL.mult, op1=AL.add)
                    continue
                if l == 0:
                    at = pab.tile([128, 1920], f32, tag="A", name="at")
                    bt = pab.tile([128, 1920], f32, tag="B", name="bt")
                    for side, dst in ((0, at), (1, bt)):
                        nc.gpsimd.indirect_dma_start(
                            out=dst[:], out_offset=None, in_=xsrc_d[:],
                            in_offset=bass.IndirectOffsetOnAxis(
                                ap=offs_t[:, 2 * key + side:
                                          2 * key + side + 1], axis=0))
                    a_ap, b_ap = raw_view(at), raw_view(bt)
                elif l < 5:
                    a_ap = shp(tiles[(l - 1, 2 * key)][:])
                    b_ap = shp(tiles[(l - 1, 2 * key + 1)][:])
                elif l == 5:
                    a_ap = shp(tiles[(4, 0)][:])
                    b_ap = shp(tiles[(4, 1)][:])
                else:
                    a_ap = shp(tiles['T5'][0:64, :])
                    b_ap = shp(tiles['T5b'][:])

                if base != 0:
                    p_full = ptmp.tile([128, F], f32, tag="p", name="p")
                    p_ap = shp(p_full[sl, :])
                else:
                    p_t = ptmp.tile([n, F], f32, tag="p", name="p")
                    p_ap = shp(p_t[:])
                q_ap = p_ap  # in-place: q overwrites p
                nc.scalar.activation(p_ap, a_ap, ACTF.Identity,
                                     bias=bi, scale=sc)
                nc.vector.scalar_tensor_tensor(
                    out=q_ap, in0=b_ap, scalar=be, in1=p_ap,
                    op0=AL.add, op1=AL.mult)
                if l == 5:
                    r_t = pfin.tile([128, F], f32, tag="T5", name="t5")
                    tiles['T5'] = r_t
                    r_ap = shp(r_t[:])
                elif l == 6:
                    # compute + store output in h-halves so the DMA of half 0
                    # overlaps the stt of half 1; final gamma add happens on host
                    r_t = pfin.tile([64, F], f32, tag="T6", name="t6")
                    for hh in (0, 1):
                        hs = (slice(None), slice(15 * hh, 15 * hh + 15),
                              slice(None), slice(None))
                        nc.vector.scalar_tensor_tensor(
                            out=shp(r_t[:])[hs], in0=a_ap[hs], scalar=c1,
                            in1=q_ap[hs], op0=AL.mult, op1=AL.add)
                        nc.sync.dma_start(
                            out_d[:, 900 * hh:900 * hh + 900],
                            r_t[:, 900 * hh:900 * hh + 900])
                    continue_l6 = True
                    r_ap = None
                else:
                    pool = pt0 if l == 0 else plv
                    r_t = pool.tile([128, F], f32, tag=f"T{l}",
                                    name=f"t{l}_{key}")
                    tiles[(l, key)] = r_t
                    r_ap = shp(r_t[:])
                if r_ap is not None:
                    nc.vector.scalar_tensor_tensor(
                        out=r_ap, in0=a_ap, scalar=c1, in1=q_ap,
                        op0=AL.mult, op1=AL.add)
                if l == 5:
                    t5b = pfin.tile([64, F], f32, tag="T5b", name="t5b")
                    tiles['T5b'] = t5b
                    nc.sync.dma_start(t5b[:], r_t[64:128, :])
    nc.compile()
    _BASS_CACHE['nc'] = nc
    return nc


def _prep_inputs(x, idx_a, idx_b, ws):
    coef = _coef_tables(ws)
    offs = _offset_tables(idx_a, idx_b)
    x = np.ascontiguousarray(x, dtype=np.float32)
    in_maps = []
    for core in range(NCORES):
        # b-interleaved slice: [C,H,W,B2]
        xs = x[B2 * core:B2 * core + B2].transpose(1, 2, 3, 0)
        xp = np.zeros((XPAD,), dtype=np.float32)
        xp[:B2 * C * H * W] = xs.reshape(-1)
        in_maps.append({"xsrc": xp.reshape(XPAD, 1), "offs": offs,
                        "coef": coef})
    return in_maps


def _assemble(core_outs, gamma):
    """core_outs: list of [64, F=(hh,ww,b)]; gamma [64] -> [16,64,900,1]."""
    full = np.stack(core_outs).astype(np.float32)   # [8, 64, 1800]
    full = full + gamma.astype(np.float32)[None, :, None]
    full = full.reshape(NCORES, K, P, B2)           # [core, k, p, b_local]
    full = full.transpose(0, 3, 1, 2).reshape(B, K, P, 1)
    return np.ascontiguousarray(full.astype(np.float32))


def kernel(x, idx_a, idx_b, w0, w1, w2, w3, w4, w5, w6):
    ws = [np.asarray(w, dtype=np.float32) for w in
          (w0, w1, w2, w3, w4, w5, w6)]
    x = np.asarray(x, dtype=np.float32)
    idx_a = np.asarray(idx_a, dtype=np.int32)
    idx_b = np.asarray(idx_b, dtype=np.int32)
    in_maps = _prep_inputs(x, idx_a, idx_b, ws)
    nc = _build_bass()
    from concourse.bass_utils import run_bass_kernel_spmd
    res = run_bass_kernel_spmd(nc, in_maps, core_ids=list(range(NCORES)))
    gamma = in_maps[0]["coef"][0:64, 4 * _NMIX]
    return _assemble([r["out"] for r in res.results], gamma)


def kernel_emulate(x, idx_a, idx_b, w0, w1, w2, w3, w4, w5, w6):
    """Pure-numpy emulation of the exact device schedule (debug aid)."""
    ws = [np.asarray(w, dtype=np.float32) for w in
          (w0, w1, w2, w3, w4, w5, w6)]
    in_maps = _prep_inputs(np.asarray(x, np.float32),
                           np.asarray(idx_a, np.int32),
                           np.asarray(idx_b, np.int32), ws)
    outs = [_emulate_core(m["xsrc"].reshape(-1), m["offs"], m["coef"])
            for m in in_maps]
    return _assemble(outs, in_maps[0]["coef"][0:64, 4 * _NMIX])

